# Initial kernel scaffold
#
"""STSPBlock Trainium2 kernel.

Structure (per core, batch-sharded B=16 -> 8 cores x B=2):
  partitions p = b*64 + channel for all activation tensors.
  - conv0+bn+LIF-input-scale folded into one K=37 im2col matmul
    (36 shifted-tap rows DMA'd from a DRAM zero-padded copy of x,
    row 36 = ones carrying the bias). LIF state add (1-c0)*v rides the
    same PSUM accumulation via a scaled-identity matmul, so the LIF
    membrane u lands complete in PSUM.
  - spike s = tensor_scalar(u >= 1); reset v' = (s < .5) * u (one
    scalar_tensor_tensor). avgpool via strided adds; all spatial means
    via accum_out side-outputs (free).
  - BETA=0 => S-state is just alpha each step. alpha scaling commutes
    out of the node convs: all 3 node convs read the SAME out0; alpha
    is applied by scaling the block-diag conv weights (tiny 2x
    tensor_scalar) after the per-step GAT/diffusion math produces
    alpha per (b, node) broadcast across partitions with one matmul.
  - y = sum of scaled-identity matmuls accumulated in PSUM.
All bn/LIF/sigmoid parameter folding is done host-side from the actual
input values at call time, so the kernel is fully general.
"""

import numpy as np

import concourse.bass as bass
import concourse.bacc as bacc
import concourse.mybir as mybir
from concourse.tile import TileContext
from concourse.bass_utils import run_bass_kernel_spmd

FP = mybir.dt.float32
Alu = mybir.AluOpType
Act = mybir.ActivationFunctionType

T, BFULL, CIN, H, W = 8, 16, 2, 64, 64
CO, NN, HEADS = 64, 4, 4
HP, WP = 32, 32
BC = 2                    # batch per core
NCORES = 8
EPS = 1e-5
DECAY = 0.6
HD = CO // HEADS          # 16


# ----------------------------------------------------------------- host consts
def _host_consts(conv0_w, bn0_g, bn0_b, bn0_m, bn0_v, lif0_w,
                 convs_w, bns_g, bns_b, bns_m, bns_v, lifs_w,
                 ft_w, ft_b, gat_w, gat_a, out_weights):
    f32 = np.float32
    sig = lambda z: 1.0 / (1.0 + np.exp(-z.astype(np.float64)))
    c0 = f32(sig(lif0_w))
    cn = sig(lifs_w).astype(f32)          # [3]
    ws = sig(out_weights).astype(f32)     # [4]

    s0c = (bn0_g / np.sqrt(bn0_v + EPS)).astype(f32)
    bias0 = ((bn0_b - bn0_m * s0c) * c0).astype(f32)
    W0f = (conv0_w * s0c[:, None, None, None] * c0).astype(f32)  # [64,2,3,3]

    # w0bd [37,128]: row p = dy*12+dx*4+b*2+ci ; col m = b2*64+co
    # variant 0: full conv + bias; variants 1-4: minus-garbage tap rows
    # (dx=0 / dx=2 / dy=0 / dy=2); variants 5-8: corner add-backs.
    w0bd = np.zeros((37, 9 * 128), f32)

    def put(v, dy, dx, sgn):
        for b in range(2):
            for ci in range(2):
                p = 1 + dy * 12 + dx * 4 + b * 2 + ci
                w0bd[p, v * 128 + b * 64:v * 128 + (b + 1) * 64] = \
                    sgn * W0f[:, ci, dy, dx]

    for dy in range(3):
        for dx in range(3):
            put(0, dy, dx, 1.0)
    w0bd[0, 0:64] = bias0
    w0bd[0, 64:128] = bias0
    for dy in range(3):
        put(1, dy, 0, -1.0)
        put(2, dy, 2, -1.0)
    for dx in range(3):
        put(3, 0, dx, -1.0)
        put(4, 2, dx, -1.0)
    put(5, 0, 0, 1.0)
    put(6, 0, 2, 1.0)
    put(7, 2, 0, 1.0)
    put(8, 2, 2, 1.0)

    i0 = ((1.0 - c0) * np.eye(128)).astype(f32)

    sncol = (bns_g / np.sqrt(bns_v + EPS)).astype(f32)            # [3,64]
    biasn_raw = (bns_b - bns_m * sncol).astype(f32)               # [3,64]
    # 0.25 = avgpool fold (out0_raw is the SUM of 4 spikes)
    Wf = (convs_w * sncol[:, :, None, None, None] * 0.25).astype(f32)

    # wnod [3, 9, 128, 128]: per (node, tap) block-diag lhsT over b
    wnod = np.zeros((3, 9, 128, 128), f32)
    for n in range(3):
        for dy in range(3):
            for dx in range(3):
                k = dy * 3 + dx
                blk = Wf[n, :, :, dy, dx].T    # [ci, co]
                wnod[n, k, 0:64, 0:64] = blk
                wnod[n, k, 64:128, 64:128] = blk

    in3 = np.stack([(1.0 - cn[n]) * np.eye(128) for n in range(3)]).astype(f32)
    biasn = np.concatenate([np.tile(cn[n] * biasn_raw[n], 2)
                            for n in range(3)]).reshape(1, 384).astype(f32)

    def bd(m):  # block-diag [128,128] of m.T twice ([co,ci] -> lhsT)
        z = np.zeros((128, 128), f32)
        z[0:64, 0:64] = m.T
        z[64:128, 64:128] = m.T
        return z

    ftmm = np.stack([bd(ft_w * (0.25 / 1024.0)), bd(ft_w * (1.0 / 1024.0))])
    ftb2 = np.tile(ft_b, 2).reshape(128, 1).astype(f32)
    gwbd = bd(gat_w).astype(f32)

    # ga1/ga2 [128, 8]: in p=(b, c') c'=h*16+d ; out m = b*4+h
    ga1 = np.zeros((128, 8), f32)
    ga2 = np.zeros((128, 8), f32)
    for b in range(2):
        for h in range(HEADS):
            for d in range(HD):
                ga1[b * 64 + h * 16 + d, b * 4 + h] = gat_a[h, d]
                ga2[b * 64 + h * 16 + d, b * 4 + h] = gat_a[h, HD + d]

    # ghbd [8,2]: p=(b,h) -> col b ; carries 0.5(sym)*0.25(mean h)/0.01(temp)
    ghbd = np.zeros((8, 2), f32)
    for b in range(2):
        ghbd[b * 4:(b + 1) * 4, b] = 12.5

    gbc = np.zeros((2, 128), f32)
    gbc[0, 0:64] = 1.0
    gbc[1, 64:128] = 1.0

    cnrow = np.tile(cn[None, :], (2, 1)).astype(f32)              # [2,3]

    iy = np.stack([ws[0] * 0.25 * np.eye(128),
                   ws[1] * np.eye(128),
                   ws[2] * np.eye(128),
                   ws[3] * np.eye(128)]).astype(f32)

    def cols(stk):  # [k,128,128] -> [128, k*128]
        return np.ascontiguousarray(
            np.transpose(stk, (1, 0, 2)).reshape(128, -1))

    return dict(w0bd=w0bd, i0=i0, wnod=cols(wnod.reshape(27, 128, 128)),
                in3=cols(in3), biasn=biasn,
                ftmm=cols(ftmm), ftb2=ftb2, gwbd=gwbd,
                ga1=ga1, ga2=ga2, ghbd=ghbd, gbc=gbc, cnrow=cnrow,
                iy=cols(iy))


CONST_SHAPES = dict(w0bd=(37, 9 * 128), i0=(128, 128), wnod=(128, 27 * 128),
                    in3=(128, 3 * 128), biasn=(1, 384), ftmm=(128, 2 * 128),
                    ftb2=(128, 1), gwbd=(128, 128), ga1=(128, 8), ga2=(128, 8),
                    ghbd=(8, 2), gbc=(2, 128), cnrow=(2, 3), iy=(128, 4 * 128))


# ------------------------------------------------------------------ the module
def build_nc(nt=T, yw=(0.25, 1.0, 1.0, 1.0)):
    nc = bacc.Bacc(None, target_bir_lowering=False)
    x = nc.declare_dram_parameter("x", [T, BC, CIN, H, W], FP, isOutput=False)
    cst = {k: nc.declare_dram_parameter(k, list(v), FP, isOutput=False)
           for k, v in CONST_SHAPES.items()}
    y = nc.declare_dram_parameter("y", [T, BC, CO, HP, WP], FP, isOutput=True)
    xlin = nc.dram_tensor("xlin", [T * 16384 + 256], FP)

    with TileContext(nc) as tc:
        with (
            tc.tile_pool(name="consts", bufs=1) as cpool,
            tc.tile_pool(name="state", bufs=1) as spool,
            tc.tile_pool(name="im", bufs=1) as impool,
            tc.tile_pool(name="work", bufs=2) as wpool,
            tc.tile_pool(name="sw", bufs=1) as swpool,
            tc.tile_pool(name="tiny", bufs=3) as tpool,
            tc.tile_pool(name="pconv", bufs=2, space="PSUM") as ps_conv,
            tc.tile_pool(name="pnode", bufs=2, space="PSUM") as ps_node,
            tc.tile_pool(name="py", bufs=1, space="PSUM") as ps_y,
            tc.tile_pool(name="ptiny", bufs=3, space="PSUM") as ps_tiny,
        ):
            # ---- consts to SBUF
            csb = {}
            for k, shp in CONST_SHAPES.items():
                t_ = cpool.tile(list(shp), FP, tag=k)
                nc.sync.dma_start(t_[:], cst[k][:])
                csb[k] = t_

            ones = cpool.tile([1, 512], FP, tag="ones")
            nc.vector.memset(ones[:], 1.0)
            actb = cpool.tile([128, 2], FP, tag="actb")
            nc.vector.memset(actb[:, 0:1], 0.0)
            nc.vector.memset(actb[:, 1:2], 1e-6)

            # ---- states
            v0a = spool.tile([128, 4096], FP, tag="v0a")
            v0b = spool.tile([128, 4096], FP, tag="v0b")
            vna = spool.tile([128, 3072], FP, tag="vna")
            vnb = spool.tile([128, 3072], FP, tag="vnb")
            Tt = spool.tile([128, 4], FP, tag="Tt")
            nc.vector.memset(v0a[:], 0.0)
            nc.vector.memset(vna[:], 0.0)
            nc.vector.memset(Tt[:], 0.0)

            # ---- x -> xlin (flat, 128-elem zero pad head/tail)
            zrow = cpool.tile([1, 128], FP, tag="zrow")
            nc.vector.memset(zrow[:], 0.0)
            xsb = wpool.tile([128, 1024], FP, tag="xsb")
            nc.sync.dma_start(
                xsb[:],
                bass.AP(tensor=x, offset=0, ap=[[1024, 128], [1, 1024]]))
            nc.gpsimd.dma_start(
                bass.AP(tensor=xlin, offset=0, ap=[[128, 1], [1, 128]]),
                zrow[:])
            nc.gpsimd.dma_start(
                bass.AP(tensor=xlin, offset=128 + T * 16384,
                        ap=[[128, 1], [1, 128]]),
                zrow[:])
            nc.gpsimd.dma_start(
                bass.AP(tensor=xlin, offset=128,
                        ap=[[1024, 128], [1, 1024]]),
                xsb[:])

            # ---- im2col tiles (row 0 = ones, set once; rows 1-36 streamed)
            imA = impool.tile([37, 4096], FP, tag="imA")
            imB = impool.tile([37, 4096], FP, tag="imB")
            for imt in (imA, imB):
                nc.vector.memset(imt[0:1, :], 1.0)

            def colmat(name, j):
                return csb[name][:, j * 128:(j + 1) * 128]
            ftb2ap = csb["ftb2"][:]

            for t in range(nt):
                v0o, v0n = (v0a, v0b) if t % 2 == 0 else (v0b, v0a)
                vno, vnn = (vna, vnb) if t % 2 == 0 else (vnb, vna)
                im = imA if t % 2 == 0 else imB

                # ---- im2col DMA: flat 16KB runs (3 per t, one per dy)
                for dy in range(3):
                    p0 = 1 + dy * 12
                    nc.sync.dma_start(
                        im[p0:p0 + 12, :],
                        bass.AP(tensor=xlin,
                                offset=128 + t * 16384 + (dy - 1) * 64 - 1,
                                ap=[[1, 3], [4096, 4], [1, 4096]]))

                # ---- conv0 + LIF0, 8 chunks of 512 (8 h-rows each)
                p1 = wpool.tile([128, 2048], FP, tag="p1")
                for c in range(8):
                    sl = slice(c * 512, (c + 1) * 512)
                    ps = ps_conv.tile([128, 512], FP, tag="pc")
                    Wv = lambda v: csb["w0bd"][:, v * 128:(v + 1) * 128]
                    nc.tensor.matmul(ps[:], Wv(0), im[:, sl],
                                     start=True, stop=False)
                    imc = im[:, sl]
                    # accumulate minus-garbage corrections on edge pixels
                    nc.tensor.matmul(ps[:, 0::64], Wv(1), imc[:, 0::64],
                                     start=False, stop=False,
                                     skip_group_check=True)
                    nc.tensor.matmul(ps[:, 63::64], Wv(2), imc[:, 63::64],
                                     start=False, stop=False,
                                     skip_group_check=True)
                    if c == 0:
                        nc.tensor.matmul(ps[:, 0:64], Wv(3), imc[:, 0:64],
                                         start=False, stop=False,
                                         skip_group_check=True)
                        nc.tensor.matmul(ps[:, 0:1], Wv(5), imc[:, 0:1],
                                         start=False, stop=False,
                                         skip_group_check=True)
                        nc.tensor.matmul(ps[:, 63:64], Wv(6), imc[:, 63:64],
                                         start=False, stop=False,
                                         skip_group_check=True)
                    if c == 7:
                        nc.tensor.matmul(ps[:, 448:512], Wv(4), imc[:, 448:512],
                                         start=False, stop=False,
                                         skip_group_check=True)
                        nc.tensor.matmul(ps[:, 448:449], Wv(7), imc[:, 448:449],
                                         start=False, stop=False,
                                         skip_group_check=True)
                        nc.tensor.matmul(ps[:, 511:512], Wv(8), imc[:, 511:512],
                                         start=False, stop=False,
                                         skip_group_check=True)
                    nc.tensor.matmul(ps[:], csb["i0"][:], v0o[:, sl],
                                     start=False, stop=True,
                                     skip_group_check=True)
                    s0c = wpool.tile([128, 512], FP, tag="s0c")
                    nc.vector.tensor_scalar(s0c[:], ps[:], 1.0, None, Alu.is_ge)
                    nc.vector.scalar_tensor_tensor(
                        v0n[:, sl], s0c[:], 0.5, ps[:], Alu.is_lt, Alu.mult)
                    s0r = s0c[:].rearrange("p (h w) -> p h w", h=8)
                    p1r = p1[:].rearrange("p (h w) -> p h w", h=64)
                    nc.vector.tensor_tensor(
                        p1r[:, c * 8:(c + 1) * 8, :],
                        s0r[:, :, 0::2], s0r[:, :, 1::2], Alu.add)

                # ---- pool rows + f0sum ; write into padded out0
                out0p = wpool.tile([128, 34 * 34], FP, tag="out0p")
                o0r = out0p[:].rearrange("p (h w) -> p h w", h=34)
                nc.vector.memset(o0r[:, 0, :], 0.0)
                nc.vector.memset(o0r[:, 33, :], 0.0)
                nc.vector.memset(o0r[:, 1:33, 0:1], 0.0)
                nc.vector.memset(o0r[:, 1:33, 33:34], 0.0)
                f0sum = tpool.tile([128, 1], FP, tag="f0sum")
                p1v = p1[:].rearrange("p (h w) -> p h w", h=64)
                nc.vector.tensor_tensor(
                    o0r[:, 1:33, 1:33], p1v[:, 0::2, :], p1v[:, 1::2, :],
                    Alu.add)
                nc.vector.tensor_reduce(f0sum[:], o0r[:, 1:33, 1:33],
                                        mybir.AxisListType.XY, Alu.add)

                # ---- f0 = relu(ft0 @ f0sum + ftb)
                psf0 = ps_tiny.tile([128, 1], FP, tag="gt")
                nc.tensor.matmul(psf0[:], colmat("ftmm", 0), f0sum[:],
                                 start=True, stop=True)
                f0 = tpool.tile([128, 1], FP, tag="f0")
                nc.vector.tensor_scalar(f0[:], psf0[:], ftb2ap, 0.0,
                                        Alu.add, op1=Alu.max)
                f04 = tpool.tile([128, 1], FP, tag="f04")
                nc.vector.tensor_scalar_mul(f04[:], f0[:], 0.4)

                # ---- trace row0 pre-update
                nc.vector.scalar_tensor_tensor(
                    Tt[:, 0:1], Tt[:, 0:1], DECAY, f04[:], Alu.mult, Alu.add)

                # ================= graph math =================
                def tiny(tag, p_, f_):
                    return tpool.tile([p_, f_], FP, tag=tag, name=tag)

                psg = ps_tiny.tile([128, 4], FP, tag="gt")
                nc.tensor.matmul(psg[:], csb["gwbd"][:], Tt[:],
                                 start=True, stop=True)
                hpc = tiny("hpc", 128, 4)
                nc.vector.tensor_copy(hpc[:], psg[:])

                pse1 = ps_tiny.tile([8, 4], FP, tag="gt")
                nc.tensor.matmul(pse1[:], csb["ga1"][:], hpc[:],
                                 start=True, stop=True)
                e1t = tiny("e1t", 8, 4)
                nc.vector.tensor_copy(e1t[:], pse1[:])
                pse2 = ps_tiny.tile([8, 4], FP, tag="gt")
                nc.tensor.matmul(pse2[:], csb["ga2"][:], hpc[:],
                                 start=True, stop=True)
                e2t = tiny("e2t", 8, 4)
                nc.vector.tensor_copy(e2t[:], pse2[:])

                def reap(ap_, tail):
                    dims = [list(d) for d in ap_.ap][:-1] + tail
                    return bass.AP(tensor=ap_.tensor, offset=ap_.offset,
                                   ap=dims)

                def bc_n(ap_):  # [p,4] -> free (n,m): n varies, m bcast
                    return reap(ap_, [[1, 4], [0, 4]])

                def bc_m(ap_):  # free (n,m): n bcast, m varies
                    return reap(ap_, [[0, 4], [1, 4]])

                es = tiny("es", 8, 16)
                nc.vector.tensor_tensor(es[:], bc_n(e1t[:]), bc_m(e2t[:]),
                                        Alu.add)
                es2 = tiny("es2", 8, 16)
                nc.vector.tensor_scalar_mul(es2[:], es[:], 0.2)
                el = tiny("el", 8, 16)
                nc.vector.tensor_tensor(el[:], es[:], es2[:], Alu.max)

                psE = ps_tiny.tile([2, 16], FP, tag="gt")
                nc.tensor.matmul(psE[:], csb["ghbd"][:], el[:],
                                 start=True, stop=True)
                Ec = tiny("Ec", 2, 16)
                nc.vector.tensor_copy(Ec[:], psE[:])

                def tr_nm(ap_):  # read transposed over (n,m)
                    return reap(ap_, [[1, 4], [4, 4]])

                L = tiny("L", 2, 16)
                nc.vector.tensor_tensor(L[:], Ec[:], tr_nm(Ec[:]), Alu.add)
                Lr = L[:].rearrange("p (n m) -> p n m", n=4)
                mx = tiny("mx", 2, 4)
                nc.vector.tensor_reduce(mx[:], Lr, mybir.AxisListType.X,
                                        Alu.max)
                xm = tiny("xm", 2, 16)
                nc.vector.tensor_tensor(xm[:], L[:], bc_n(mx[:]), Alu.subtract)
                ex = tiny("ex", 2, 16)
                nc.scalar.activation(ex[:], xm[:], Act.Exp,
                                     bias=actb[0:2, 0:1])
                sm = tiny("sm", 2, 4)
                exr = ex[:].rearrange("p (n m) -> p n m", n=4)
                nc.vector.tensor_reduce(sm[:], exr, mybir.AxisListType.X,
                                        Alu.add)
                rc = tiny("rc", 2, 4)
                nc.vector.reciprocal(rc[:], sm[:])
                S = tiny("S", 2, 16)
                nc.vector.tensor_tensor(S[:], ex[:], bc_n(rc[:]), Alu.mult)

                Sr = S[:].rearrange("p (n m) -> p n m", n=4)
                lo = tiny("lo", 2, 8)
                lor = lo[:].rearrange("p (n m) -> p n m", n=4)
                hi = tiny("hi", 2, 8)
                hir = hi[:].rearrange("p (n m) -> p n m", n=4)
                nc.vector.tensor_tensor(lor, Sr[:, :, 0::2], Sr[:, :, 1::2],
                                        Alu.min)
                nc.vector.tensor_tensor(hir, Sr[:, :, 0::2], Sr[:, :, 1::2],
                                        Alu.max)
                kth = tiny("kth", 2, 4)
                l2 = tiny("l2", 2, 4)
                nc.vector.tensor_tensor(l2[:], lor[:, :, 0], lor[:, :, 1],
                                        Alu.max)
                h2 = tiny("h2", 2, 4)
                nc.vector.tensor_tensor(h2[:], hir[:, :, 0], hir[:, :, 1],
                                        Alu.min)
                nc.vector.tensor_tensor(kth[:], l2[:], h2[:], Alu.min)
                msk = tiny("msk", 2, 16)
                nc.vector.tensor_tensor(msk[:], S[:], bc_n(kth[:]), Alu.is_ge)
                Sp = tiny("Sp", 2, 16)
                nc.vector.tensor_tensor(Sp[:], S[:], msk[:], Alu.mult)

                A2 = tiny("A2", 2, 16)
                nc.vector.tensor_tensor(A2[:], Sp[:], tr_nm(Sp[:]), Alu.add)
                rs = tiny("rs", 2, 4)
                A2r = A2[:].rearrange("p (n m) -> p n m", n=4)
                nc.vector.tensor_reduce(rs[:], A2r, mybir.AxisListType.X,
                                        Alu.add)
                lnd = tiny("lnd", 2, 4)
                nc.scalar.activation(lnd[:], rs[:], Act.Ln,
                                     bias=actb[0:2, 1:2], scale=0.5)
                q = tiny("q", 2, 4)
                nc.scalar.activation(q[:], lnd[:], Act.Exp, scale=-0.5,
                                     bias=actb[0:2, 0:1])

                t1 = tiny("t1", 2, 16)
                nc.vector.tensor_tensor(t1[:], A2[:], bc_n(q[:]), Alu.mult)
                OPt = tiny("OPt", 2, 16)
                nc.vector.scalar_tensor_tensor(OPt[:], t1[:], 0.5, bc_m(q[:]),
                                               Alu.mult, Alu.mult)
                col0 = reap(OPt[:], [[0, 4], [4, 4]])
                t2 = tiny("t2", 2, 16)
                nc.vector.tensor_tensor(t2[:], OPt[:], col0, Alu.mult)
                af = tiny("af", 2, 4)
                t2r = t2[:].rearrange("p (n m) -> p n m", n=4)
                nc.vector.tensor_reduce(af[:], t2r, mybir.AxisListType.X,
                                        Alu.add)
                al3 = tiny("al3", 2, 3)
                nc.vector.tensor_tensor(al3[:], af[:, 1:4], csb["cnrow"][:],
                                        Alu.mult)
                psb = ps_tiny.tile([128, 3], FP, tag="gt")
                nc.tensor.matmul(psb[:], csb["gbc"][:], al3[:],
                                 start=True, stop=True)
                aap = tiny("aap", 128, 3)
                nc.vector.tensor_copy(aap[:], psb[:])

                # ================= node path =================
                sn = wpool.tile([128, 3072], FP, tag="sn")
                snsum = tpool.tile([128, 6], FP, tag="snsum")
                sw = [swpool.tile([128, 9 * 128], FP, tag=f"sw{n}",
                                  name=f"sw{n}") for n in range(3)]
                for n in range(3):
                    nc.vector.tensor_scalar_mul(
                        sw[n][:],
                        csb["wnod"][:, n * 9 * 128:(n + 1) * 9 * 128],
                        aap[:, n:n + 1])
                for n in range(3):
                    for c in range(2):
                        psn = ps_node.tile([128, 512], FP, tag="pn")
                        for k in range(9):
                            dy, dx = k // 3, k % 3
                            rhs = o0r[:, dy + 16 * c: dy + 16 * c + 16,
                                      dx:dx + 32]
                            nc.tensor.matmul(psn[:],
                                             sw[n][:, k * 128:(k + 1) * 128],
                                             rhs, start=(k == 0), stop=False)
                        nc.tensor.matmul(
                            psn[:], csb["biasn"][0:1, n * 128:(n + 1) * 128],
                            ones[:], start=False, stop=False)
                        nc.tensor.matmul(
                            psn[:], colmat("in3", n),
                            vno[:, n * 1024 + c * 512: n * 1024 + (c + 1) * 512],
                            start=False, stop=True)
                        sl = slice(n * 1024 + c * 512, n * 1024 + (c + 1) * 512)
                        nc.vector.tensor_scalar(
                            sn[:, sl], psn[:], 1.0, 0.0, Alu.is_ge,
                            op1=Alu.add,
                            accum_out=snsum[:, n * 2 + c: n * 2 + c + 1])
                        nc.vector.scalar_tensor_tensor(
                            vnn[:, sl], sn[:, sl], 0.5, psn[:],
                            Alu.is_lt, Alu.mult)

                # ---- feats + trace update
                psf = ps_tiny.tile([128, 3], FP, tag="gt")
                nc.tensor.matmul(psf[:], colmat("ftmm", 1), snsum[:, 0::2],
                                 start=True, stop=False)
                nc.tensor.matmul(psf[:], colmat("ftmm", 1), snsum[:, 1::2],
                                 start=False, stop=True)
                fn = tpool.tile([128, 3], FP, tag="fn")
                nc.vector.tensor_scalar(fn[:], psf[:], ftb2ap, 0.0,
                                        Alu.add, op1=Alu.max)
                fn04 = tpool.tile([128, 3], FP, tag="fn04")
                nc.vector.tensor_scalar_mul(fn04[:], fn[:], 0.4)
                nc.vector.scalar_tensor_tensor(
                    Tt[:, 0:1], Tt[:, 0:1], DECAY, f04[:], Alu.mult, Alu.add)
                nc.vector.scalar_tensor_tensor(
                    Tt[:, 1:4], Tt[:, 1:4], DECAY, fn04[:], Alu.mult, Alu.add)

                # ================= output y (DVE) =================
                ysb = wpool.tile([128, 1024], FP, tag="ysb")
                nc.vector.tensor_scalar_mul(ysb[:], o0r[:, 1:33, 1:33],
                                            yw[0])
                for n in range(3):
                    nc.vector.scalar_tensor_tensor(
                        ysb[:], sn[:, n * 1024:(n + 1) * 1024], yw[n + 1],
                        ysb[:], Alu.mult, Alu.add)
                nc.sync.dma_start(
                    bass.AP(tensor=y, offset=t * BC * CO * 1024,
                            ap=[[1024, 128], [1, 1024]]),
                    ysb[:])
    if not nc.is_finalized():
        nc.finalize()
    return nc


_NC_CACHE = {}


def _get_nc(nt=T, yw=(0.25, 1.0, 1.0, 1.0)):
    key = (nt, tuple(float(v) for v in yw))
    if key not in _NC_CACHE:
        _NC_CACHE[key] = build_nc(nt, yw)
    return _NC_CACHE[key]


def kernel(**inputs):
    x = np.asarray(inputs["x"], np.float32)
    consts = _host_consts(
        inputs["conv0_w"], inputs["bn0_g"], inputs["bn0_b"], inputs["bn0_m"],
        inputs["bn0_v"], inputs["lif0_w"], inputs["convs_w"], inputs["bns_g"],
        inputs["bns_b"], inputs["bns_m"], inputs["bns_v"], inputs["lifs_w"],
        inputs["ft_w"], inputs["ft_b"], inputs["gat_w"], inputs["gat_a"],
        inputs["out_weights"])
    consts = {k: np.ascontiguousarray(v, np.float32)
              for k, v in consts.items()}
    sigw = 1.0 / (1.0 + np.exp(-np.asarray(inputs["out_weights"], np.float64)))
    yw = (float(sigw[0]) * 0.25, float(sigw[1]), float(sigw[2]),
          float(sigw[3]))
    nc = _get_nc(T, yw)
    core_ids = list(range(NCORES))
    in_maps = []
    for k in core_ids:
        m = dict(consts)
        m["x"] = np.ascontiguousarray(x[:, k * BC:(k + 1) * BC])
        in_maps.append(m)
    res = run_bass_kernel_spmd(nc, in_maps, core_ids).results
    out = np.concatenate([res[k]["y"] for k in core_ids], axis=1)
    return out.astype(np.float32)



# revision 3
# speedup vs baseline: 1.0639x; 1.0639x over previous
"""STSPBlock Trainium2 kernel, v2.

Per core (batch-sharded B=16 -> 8 cores x B=2), partitions p = b*64+c.

Key points vs v1:
  - conv0 + node convs run in bf16 (1 cyc/row vs 4 for fp32); the LIF
    state-add identity matmuls run in float32r (full rate at N=512, v
    stays fp32 in SBUF).
  - x is laid out host-side as zero-padded 66x66 bf16 blocks in DRAM,
    so the K=37 im2col matmul needs no edge corrections at all.
  - spikes are computed on the ACT engine as sign(u-1) in {-1,+1};
    all consumers are linear, so the affine (s+1)/2 correction is
    folded host-side into weights/biases. Membrane reset runs on DVE,
    2x2 pooling + y-assembly + alpha weight-scaling run on GPSIMD.
  - pool output is converted back to true spike-sum domain (one DVE
    op with accum_out giving f0sum for free), so node convs see the
    same rhs as v1.
  - bias matmuls are emitted only when the folded biases are nonzero
    (they are zero for this model's BN/ft parameters).
  - graph-attention chain shortened: gat_w @ gat_a folded host-side.
  - Exp/Ln/Sign/Relu all live in one HW activation table (no reloads).
"""

import numpy as np
import ml_dtypes

import concourse.bass as bass
import concourse.bacc as bacc
import concourse.mybir as mybir
from concourse.tile import TileContext
from concourse.bass_utils import run_bass_kernel_spmd

FP = mybir.dt.float32
FR = mybir.dt.float32r
BF = mybir.dt.bfloat16
Alu = mybir.AluOpType
Act = mybir.ActivationFunctionType

T, BFULL, CIN, H, W = 8, 16, 2, 64, 64
CO, NN, HEADS = 64, 4, 4
HP, WP = 32, 32
BC = 2
NCORES = 8
EPS = 1e-5
DECAY = 0.6
HD = CO // HEADS
XPT = 72 * 4096         # host-im2col rows (hi 36 + lo 36) per step


# ----------------------------------------------------------------- host consts
def _host_consts(conv0_w, bn0_g, bn0_b, bn0_m, bn0_v, lif0_w,
                 convs_w, bns_g, bns_b, bns_m, bns_v, lifs_w,
                 ft_w, ft_b, gat_w, gat_a, out_weights):
    f32 = np.float32
    bf = ml_dtypes.bfloat16
    sig = lambda z: 1.0 / (1.0 + np.exp(-np.asarray(z, np.float64)))
    c0 = f32(sig(lif0_w))
    cn = sig(lifs_w).astype(f32)          # [3]
    ws = sig(out_weights).astype(f32)     # [4]

    s0c = (bn0_g / np.sqrt(bn0_v + EPS)).astype(f32)
    bias0 = ((bn0_b - bn0_m * s0c) * c0).astype(f32)
    W0f = (conv0_w * s0c[:, None, None, None] * c0).astype(f32)  # [64,2,3,3]

    # conv0 folded lhsT [109, 128], split-bf16 via spare K rows:
    #   rows 1-36:  Whi (pairs with xhi), rows 37-72: Wlo (pairs with xhi),
    #   rows 73-108: Whi (pairs with xlo); row 0 = bias (ones row).
    # Dropped Wlo*xlo term is O(2^-16) relative.
    W0hi = W0f.astype(bf).astype(f32)
    W0lo = (W0f - W0hi).astype(bf).astype(f32)
    w0bd = np.zeros((109, 128), f32)
    for dy in range(3):
        for dx in range(3):
            for b in range(2):
                for ci in range(2):
                    p = 1 + dy * 12 + dx * 4 + b * 2 + ci
                    w0bd[p, b * 64:(b + 1) * 64] = W0hi[:, ci, dy, dx]
                    w0bd[36 + p, b * 64:(b + 1) * 64] = W0lo[:, ci, dy, dx]
                    w0bd[72 + p, b * 64:(b + 1) * 64] = W0hi[:, ci, dy, dx]
    w0bd[0, 0:64] = bias0
    w0bd[0, 64:128] = bias0

    sncol = (bns_g / np.sqrt(bns_v + EPS)).astype(f32)            # [3,64]
    biasn_raw = (bns_b - bns_m * sncol).astype(f32)               # [3,64]
    # 0.125 = avgpool fold (out0p holds 2x the SUM of 4 spikes)
    Wf = (convs_w * sncol[:, :, None, None, None] * 0.125).astype(f32)

    wnod = np.zeros((3, 9, 128, 128), f32)
    for n in range(3):
        for dy in range(3):
            for dx in range(3):
                k = dy * 3 + dx
                blk = Wf[n, :, :, dy, dx].T    # [ci, co]
                wnod[n, k, 0:64, 0:64] = blk
                wnod[n, k, 64:128, 64:128] = blk

    biasn = np.concatenate([np.tile(cn[n] * biasn_raw[n], 2)
                            for n in range(3)]).reshape(1, 384).astype(f32)

    def bd(m):  # block-diag [128,128] of m.T twice ([co,ci] -> lhsT)
        z = np.zeros((128, 128), f32)
        z[0:64, 0:64] = m.T
        z[64:128, 64:128] = m.T
        return z

    # f0: pre-relu = ft_w @ (f0sum/4096) + ft_b          (f0sum: true sums)
    # fn: pre-relu = ft_w @ (snsum/2048) + ft_b + 0.5*ft_w.sum(1)
    #     (snsum = sum over pixels of sign values in {-1,+1})
    ftmm = np.stack([bd(ft_w * (1.0 / 8192.0)), bd(ft_w * (1.0 / 2048.0))])
    ftb0 = np.tile(ft_b, 2).reshape(128, 1).astype(f32)
    ftb1 = np.tile(ft_b + 0.5 * ft_w.sum(axis=1), 2).reshape(128, 1).astype(f32)

    gwbd = bd(gat_w).astype(f32)
    ga1 = np.zeros((128, 8), f32)
    ga2 = np.zeros((128, 8), f32)
    for b in range(2):
        for h in range(HEADS):
            for d in range(HD):
                ga1[b * 64 + h * 16 + d, b * 4 + h] = gat_a[h, d]
                ga2[b * 64 + h * 16 + d, b * 4 + h] = gat_a[h, HD + d]
    M1 = (gwbd @ ga1).astype(f32)          # [128, 8] folded gat_w @ a_l
    M2 = (gwbd @ ga2).astype(f32)

    ghbd = np.zeros((8, 2), f32)
    for b in range(2):
        ghbd[b * 4:(b + 1) * 4, b] = 12.5

    gbc = np.zeros((2, 128), f32)
    gbc[0, 0:64] = 1.0
    gbc[1, 64:128] = 1.0

    cnrow = np.tile(cn[None, :], (2, 1)).astype(f32)              # [2,3]

    def cols(stk):  # [k,128,128] -> [128, k*128]
        return np.ascontiguousarray(
            np.transpose(stk, (1, 0, 2)).reshape(128, -1))

    consts = dict(
        w0bd=w0bd.astype(bf),
        wnod=cols(wnod.reshape(27, 128, 128)).astype(bf),
        biasn=biasn,
        ftmm=cols(ftmm), ftb0=ftb0, ftb1=ftb1,
        m1=M1, m2=M2, ghbd=ghbd, gbc=gbc, cnrow=cnrow)

    # y scales: y = ws0*(out0sum/4) + sum_n wsn*(sn_sign+1)/2
    # plus LIF leak factors for the on-chip u-ops
    yw = (float(ws[0]) * 0.125,
          float(ws[1]) * 0.5, float(ws[2]) * 0.5, float(ws[3]) * 0.5,
          float(0.5 * (ws[1] + ws[2] + ws[3])),
          float(1.0 - c0), float(1.0 - cn[0]), float(1.0 - cn[1]),
          float(1.0 - cn[2]))
    flags = (bool(np.any(np.abs(biasn) > 0)),)
    return consts, yw, flags


CONST_SPECS = dict(w0bd=((109, 128), BF),
                   wnod=((128, 27 * 128), BF),
                   biasn=((1, 384), FP), ftmm=((128, 2 * 128), FP),
                   ftb0=((128, 1), FP), ftb1=((128, 1), FP),
                   m1=((128, 8), FP), m2=((128, 8), FP),
                   ghbd=((8, 2), FP), gbc=((2, 128), FP),
                   cnrow=((2, 3), FP))


# ------------------------------------------------------------------ the module
DBG = False


def build_nc(yw, biasn_nz):
    nc = bacc.Bacc(None, target_bir_lowering=False)
    xpad = nc.declare_dram_parameter("xpad", [T * XPT], BF, isOutput=False)
    cst = {k: nc.declare_dram_parameter(k, list(shp), dt, isOutput=False)
           for k, (shp, dt) in CONST_SPECS.items()}
    y = nc.declare_dram_parameter("y", [T, BC, CO, HP, WP], FP, isOutput=True)
    dbg = {}
    if DBG:
        for nm, shp, dt in [("d_s0", [128, 4096], BF), ("d_o0", [128, 1156], BF),
                        ("d_f0sum", [128, 1], FP), ("d_f0t", [128, 1], FP),
                        ("d_Tt", [128, 4], FP), ("d_es", [8, 16], FP),
                        ("d_S", [2, 16], FP), ("d_aap", [128, 3], FP),
                        ("d_q", [2, 4], FP), ("d_s1", [128, 3072], BF),
                        ("d_snsum", [128, 6], FP), ("d_u1", [128, 4096], FP)]:
            dbg[nm] = nc.declare_dram_parameter(nm, shp, dt, isOutput=True)

    w0s4, w1, w2, w3, yC, l0, ln1, ln2, ln3 = yw
    wns = (w1, w2, w3)
    lns = (ln1, ln2, ln3)

    with TileContext(nc) as tc:
        with (
            tc.tile_pool(name="consts", bufs=1) as cpool,
            tc.tile_pool(name="state", bufs=1) as spool,
            tc.tile_pool(name="im", bufs=2) as impool,
            tc.tile_pool(name="big", bufs=2) as bpool,
            tc.tile_pool(name="sw", bufs=2) as swpool,
            tc.tile_pool(name="tiny", bufs=4) as tpool,
            tc.tile_pool(name="pconv", bufs=3, space="PSUM") as ps_conv,
            tc.tile_pool(name="pnode", bufs=3, space="PSUM") as ps_node,
            tc.tile_pool(name="ptiny", bufs=1, space="PSUM") as ps_tiny,
            tc.tile_pool(name="ptiny2", bufs=1, space="PSUM") as ps_tiny2,
        ):
            # ---- consts to SBUF (w0bd first on sync; rest on ACT queue
            # so the first conv0 + im DMAs aren't stuck behind them)
            csb = {}
            for k, (shp, dt) in CONST_SPECS.items():
                t_ = cpool.tile(list(shp), dt, tag=k)
                (nc.sync if k == "w0bd" else nc.scalar).dma_start(
                    t_[:], cst[k][:])
                csb[k] = t_

            ones = None
            if biasn_nz:
                ones = cpool.tile([1, 512], FP, tag="ones")
                nc.vector.memset(ones[:], 1.0)

            # activation biases must be APs: [-1.0 (sign), 1e-6 (ln), 0.0]
            actc = cpool.tile([128, 3], FP, tag="actc")
            nc.vector.memset(actc[:, 0:1], -1.0)
            nc.vector.memset(actc[:, 1:2], 1e-6)
            nc.vector.memset(actc[:, 2:3], 0.0)

            # ---- states (in-place: u-op reads v before reset rewrites it)
            v0 = spool.tile([128, 4096], FP, tag="v0")
            vn = spool.tile([128, 3072], FP, tag="vn")
            u0t = spool.tile([128, 4096], FP, tag="u0t")
            unt = spool.tile([128, 3072], FP, tag="unt")
            Tt = spool.tile([128, 4], FP, tag="Tt")
            nc.vector.memset(Tt[:], 0.0)

            # out0p (true spike-sum domain, zero border = zero pad)
            o0A = spool.tile([128, 34 * 34], BF, tag="o0A")
            o0B = spool.tile([128, 34 * 34], BF, tag="o0B")
            nc.gpsimd.memset(o0A[:], 0.0)
            nc.gpsimd.memset(o0B[:], 0.0)

            imA = impool.tile([109, 4096], BF, tag="imA")
            imB = impool.tile([109, 4096], BF, tag="imB")
            for imt in (imA, imB):
                nc.vector.memset(imt[0:1, :], 1.0)

            def im_dma(t_, imt):
                hi = bass.AP(tensor=xpad, offset=t_ * XPT,
                             ap=[[4096, 36], [1, 4096]])
                lo = bass.AP(tensor=xpad, offset=t_ * XPT + 36 * 4096,
                             ap=[[4096, 36], [1, 4096]])
                nc.sync.dma_start(imt[1:37, :], hi)
                nc.sync.dma_start(imt[37:73, :], hi)
                nc.sync.dma_start(imt[73:109, :], lo)

            def colmat(name, j, w=128):
                return csb[name][:, j * w:(j + 1) * w]

            im_dma(0, imA)

            # persistent-ish per-step tiles come from rotating pools
            prev = None  # state carried from step t-1 for the node path

            for t in range(T + 1):
                if t < T:
                    im = imA if t % 2 == 0 else imB
                    o0 = o0A if t % 2 == 0 else o0B
                    o0r = o0[:].rearrange("p (h w) -> p h w", h=34)

                # ========== conv0(t): matmuls + drains ==========
                if t < T:
                    s0t = bpool.tile([128, 4096], BF, tag="s0t")
                    p1 = bpool.tile([128, 2048], BF, tag="p1")
                    p1r = p1[:].rearrange("p (h w) -> p h w", h=64)
                    for c in range(8):
                        sl = slice(c * 512, (c + 1) * 512)
                        ps = ps_conv.tile([128, 512], FP, tag="pc")
                        nc.tensor.matmul(ps[:], csb["w0bd"][:], im[:, sl],
                                         start=True, stop=True)
                        if t == 0:
                            # no membrane yet: u == ps (skips v0/u0t init)
                            nc.scalar.activation(s0t[:, sl], ps[:], Act.Sign,
                                                 bias=actc[:, 0:1])
                            nc.vector.scalar_tensor_tensor(
                                v0[:, sl], s0t[:, sl], 0.0, ps[:],
                                Alu.is_lt, Alu.mult)
                        else:
                            # u = (1-c0)*v + conv  (exact, on DVE)
                            nc.vector.scalar_tensor_tensor(
                                u0t[:, sl], v0[:, sl], l0, ps[:],
                                Alu.mult, Alu.add)
                            nc.scalar.activation(s0t[:, sl], u0t[:, sl],
                                                 Act.Sign, bias=actc[:, 0:1])
                            nc.gpsimd.scalar_tensor_tensor(
                                v0[:, sl], s0t[:, sl], 0.0, u0t[:, sl],
                                Alu.is_lt, Alu.mult)
                        s0r = s0t[:, sl].rearrange("p (h w) -> p h w", h=8)
                        nc.gpsimd.tensor_tensor(
                            p1r[:, c * 8:(c + 1) * 8, :],
                            s0r[:, :, 0::2], s0r[:, :, 1::2], Alu.add)

                    if t + 1 < T:
                        im_dma(t + 1, imB if t % 2 == 0 else imA)

                    # pool-V + back to true-sum domain (+f0sum for free)
                    pv = bpool.tile([128, 1024], BF, tag="pv")
                    nc.gpsimd.tensor_tensor(
                        pv[:], p1r[:, 0::2, :], p1r[:, 1::2, :], Alu.add)
                    f0sum = tpool.tile([128, 1], FP, tag="f0sum")
                    pvr = pv[:].rearrange("p (h w) -> p h w", h=32)
                    nc.vector.tensor_scalar(
                        o0r[:, 1:33, 1:33], pvr, 4.0, None, Alu.add,
                        op1=Alu.add, accum_out=f0sum[:])

                    if DBG and t == 0:
                        nc.sync.dma_start(dbg["d_s0"][:], s0t[:])
                        nc.sync.dma_start(dbg["d_o0"][:], o0[:])
                        nc.sync.dma_start(dbg["d_f0sum"][:], f0sum[:])
                    if DBG and t == 1:
                        nc.sync.dma_start(dbg["d_u1"][:], u0t[:])
                    # f0 = relu(ft @ f0sum/4096 + ftb)
                    psf0 = ps_tiny.tile([128, 1], FP, tag="gt")
                    nc.tensor.matmul(psf0[:], colmat("ftmm", 0), f0sum[:],
                                     start=True, stop=True)
                    f0t = tpool.tile([128, 1], FP, tag="f0t")
                    nc.vector.tensor_scalar(f0t[:], psf0[:], csb["ftb0"][:],
                                            0.0, Alu.add, op1=Alu.max)
                    f04 = tpool.tile([128, 1], FP, tag="f04")
                    nc.vector.tensor_scalar_mul(f04[:], f0t[:], 0.4)

                # ========== node path for t-1 ==========
                if prev is not None:
                    po0r, psw, pf04, pt = prev
                    s1t = bpool.tile([128, 3072], BF, tag="s1t")
                    snsum = tpool.tile([128, 6], FP, tag="snsum")
                    for n in range(3):
                        for c in range(2):
                            psn = ps_node.tile([128, 512], FP, tag="pn")
                            for k in range(9):
                                dy, dx = k // 3, k % 3
                                rhs = po0r[:, dy + 16 * c: dy + 16 * c + 16,
                                           dx:dx + 32]
                                nc.tensor.matmul(
                                    psn[:], psw[n][:, k * 128:(k + 1) * 128],
                                    rhs, start=(k == 0),
                                    stop=(k == 8 and not biasn_nz))
                            if biasn_nz:
                                nc.tensor.matmul(
                                    psn[:],
                                    csb["biasn"][0:1, n * 128:(n + 1) * 128],
                                    ones[:], start=False, stop=True)
                            sl = slice(n * 1024 + c * 512,
                                       n * 1024 + (c + 1) * 512)
                            if pt == 0:
                                nc.scalar.activation(
                                    s1t[:, sl], psn[:], Act.Sign,
                                    bias=actc[:, 0:1],
                                    accum_out=snsum[:, n * 2 + c:
                                                    n * 2 + c + 1])
                                nc.vector.scalar_tensor_tensor(
                                    vn[:, sl], s1t[:, sl], 0.0, psn[:],
                                    Alu.is_lt, Alu.mult)
                            else:
                                nc.vector.scalar_tensor_tensor(
                                    unt[:, sl], vn[:, sl], lns[n], psn[:],
                                    Alu.mult, Alu.add)
                                nc.scalar.activation(
                                    s1t[:, sl], unt[:, sl], Act.Sign,
                                    bias=actc[:, 0:1],
                                    accum_out=snsum[:, n * 2 + c:
                                                    n * 2 + c + 1])
                                nc.gpsimd.scalar_tensor_tensor(
                                    vn[:, sl], s1t[:, sl], 0.0, unt[:, sl],
                                    Alu.is_lt, Alu.mult)

                    if DBG and pt == 0:
                        nc.sync.dma_start(dbg["d_s1"][:], s1t[:])
                        nc.sync.dma_start(dbg["d_snsum"][:], snsum[:])
                    # ---- y(t-1) on GPSIMD + DMA out
                    ysb = bpool.tile([128, 1024], FP, tag="ysb")
                    nc.gpsimd.tensor_scalar(
                        ysb[:], po0r[:, 1:33, 1:33], w0s4, yC,
                        Alu.mult, op1=Alu.add)
                    for n in range(3):
                        nc.gpsimd.scalar_tensor_tensor(
                            ysb[:], s1t[:, n * 1024:(n + 1) * 1024], wns[n],
                            ysb[:], Alu.mult, Alu.add)
                    nc.sync.dma_start(
                        bass.AP(tensor=y, offset=(t - 1) * BC * CO * 1024,
                                ap=[[1024, 128], [1, 1024]]),
                        ysb[:])

                    # ---- feats(t-1) + full trace update
                    if t >= T:
                        break
                    psf = ps_tiny.tile([128, 3], FP, tag="gt")
                    nc.tensor.matmul(psf[:], colmat("ftmm", 1),
                                     snsum[:, 0::2], start=True, stop=False)
                    nc.tensor.matmul(psf[:], colmat("ftmm", 1),
                                     snsum[:, 1::2], start=False, stop=True)
                    fnt = tpool.tile([128, 3], FP, tag="fnt")
                    nc.vector.tensor_scalar(fnt[:], psf[:], csb["ftb1"][:],
                                            0.0, Alu.add, op1=Alu.max)
                    fn04 = tpool.tile([128, 3], FP, tag="fn04")
                    nc.vector.tensor_scalar_mul(fn04[:], fnt[:], 0.4)
                    nc.vector.scalar_tensor_tensor(
                        Tt[:, 0:1], Tt[:, 0:1], DECAY, pf04[:],
                        Alu.mult, Alu.add)
                    nc.vector.scalar_tensor_tensor(
                        Tt[:, 1:4], Tt[:, 1:4], DECAY, fn04[:],
                        Alu.mult, Alu.add)

                if t >= T:
                    break

                # trace row-0 pre-update with f0(t)
                nc.vector.scalar_tensor_tensor(
                    Tt[:, 0:1], Tt[:, 0:1], DECAY, f04[:], Alu.mult, Alu.add)
                if DBG and t == 0:
                    nc.sync.dma_start(dbg["d_f0t"][:], f0t[:])
                    nc.sync.dma_start(dbg["d_Tt"][:], Tt[:])

                # ========== graph math (t) ==========
                def tiny(tag, p_, f_):
                    return tpool.tile([p_, f_], FP, tag=tag, name=tag)

                pse1 = ps_tiny.tile([8, 4], FP, tag="gt")
                nc.tensor.matmul(pse1[:], csb["m1"][:], Tt[:],
                                 start=True, stop=True)
                pse2 = ps_tiny2.tile([8, 4], FP, tag="gt2")
                nc.tensor.matmul(pse2[:], csb["m2"][:], Tt[:],
                                 start=True, stop=True)

                def reap(ap_, tail):
                    dims = [list(d) for d in ap_.ap][:-1] + tail
                    return bass.AP(tensor=ap_.tensor, offset=ap_.offset,
                                   ap=dims)

                def bc_n(ap_):  # [p,4] -> free (n,m): n varies, m bcast
                    return reap(ap_, [[1, 4], [0, 4]])

                def bc_m(ap_):  # free (n,m): n bcast, m varies
                    return reap(ap_, [[0, 4], [1, 4]])

                e2t = tiny("e2t", 8, 4)
                nc.vector.tensor_copy(e2t[:], pse2[:])
                es = tiny("es", 8, 16)
                nc.vector.tensor_tensor(es[:], bc_n(pse1[:]), bc_m(e2t[:]),
                                        Alu.add)
                es2 = tiny("es2", 8, 16)
                nc.vector.tensor_scalar_mul(es2[:], es[:], 0.2)
                el = tiny("el", 8, 16)
                nc.vector.tensor_tensor(el[:], es[:], es2[:], Alu.max)

                if DBG and t == 0:
                    nc.sync.dma_start(dbg["d_es"][:], es[:])
                psE = ps_tiny.tile([2, 16], FP, tag="gt")
                nc.tensor.matmul(psE[:], csb["ghbd"][:], el[:],
                                 start=True, stop=True)
                Ec = tiny("Ec", 2, 16)
                nc.vector.tensor_copy(Ec[:], psE[:])

                def tr_nm(ap_):  # read transposed over (n,m)
                    return reap(ap_, [[1, 4], [4, 4]])

                L = tiny("L", 2, 16)
                nc.vector.tensor_tensor(L[:], Ec[:], tr_nm(Ec[:]), Alu.add)
                Lr = L[:].rearrange("p (n m) -> p n m", n=4)
                mx = tiny("mx", 2, 4)
                nc.vector.tensor_reduce(mx[:], Lr, mybir.AxisListType.X,
                                        Alu.max)
                xm = tiny("xm", 2, 16)
                nc.vector.tensor_tensor(xm[:], L[:], bc_n(mx[:]), Alu.subtract)
                ex = tiny("ex", 2, 16)
                nc.scalar.activation(ex[:], xm[:], Act.Exp,
                                     bias=actc[0:2, 2:3])
                sm = tiny("sm", 2, 4)
                exr = ex[:].rearrange("p (n m) -> p n m", n=4)
                nc.vector.tensor_reduce(sm[:], exr, mybir.AxisListType.X,
                                        Alu.add)
                rc = tiny("rc", 2, 4)
                nc.vector.reciprocal(rc[:], sm[:])
                S = tiny("S", 2, 16)
                nc.vector.tensor_tensor(S[:], ex[:], bc_n(rc[:]), Alu.mult)

                if DBG and t == 0:
                    nc.sync.dma_start(dbg["d_S"][:], S[:])
                Sr = S[:].rearrange("p (n m) -> p n m", n=4)
                lo = tiny("lo", 2, 8)
                lor = lo[:].rearrange("p (n m) -> p n m", n=4)
                hi = tiny("hi", 2, 8)
                hir = hi[:].rearrange("p (n m) -> p n m", n=4)
                nc.vector.tensor_tensor(lor, Sr[:, :, 0::2], Sr[:, :, 1::2],
                                        Alu.min)
                nc.vector.tensor_tensor(hir, Sr[:, :, 0::2], Sr[:, :, 1::2],
                                        Alu.max)
                kth = tiny("kth", 2, 4)
                l2 = tiny("l2", 2, 4)
                nc.vector.tensor_tensor(l2[:], lor[:, :, 0], lor[:, :, 1],
                                        Alu.max)
                h2 = tiny("h2", 2, 4)
                nc.vector.tensor_tensor(h2[:], hir[:, :, 0], hir[:, :, 1],
                                        Alu.min)
                nc.vector.tensor_tensor(kth[:], l2[:], h2[:], Alu.min)
                msk = tiny("msk", 2, 16)
                nc.vector.tensor_tensor(msk[:], S[:], bc_n(kth[:]), Alu.is_ge)
                Sp = tiny("Sp", 2, 16)
                nc.vector.tensor_tensor(Sp[:], S[:], msk[:], Alu.mult)

                A2 = tiny("A2", 2, 16)
                nc.vector.tensor_tensor(A2[:], Sp[:], tr_nm(Sp[:]), Alu.add)
                rs = tiny("rs", 2, 4)
                A2r = A2[:].rearrange("p (n m) -> p n m", n=4)
                nc.vector.tensor_reduce(rs[:], A2r, mybir.AxisListType.X,
                                        Alu.add)
                # q = rsqrt(0.5*rs + 1e-6) via bit-trick + 2 Newton iters
                U32 = mybir.dt.uint32
                zq = tiny("zq", 2, 4)
                nc.vector.tensor_scalar(zq[:], rs[:], 0.5, 1e-6, Alu.mult,
                                        op1=Alu.add)
                zsh = tiny("zsh", 2, 4)
                nc.vector.tensor_scalar(zsh[:].bitcast(U32),
                                        zq[:].bitcast(U32), 1,
                                        None, Alu.logical_shift_right)
                nc.vector.tensor_scalar(zsh[:].bitcast(U32),
                                        zsh[:].bitcast(U32), 0xFFFFFFFF,
                                        None, Alu.bitwise_xor)
                q = tiny("q", 2, 4)
                nc.vector.tensor_scalar(q[:].bitcast(U32),
                                        zsh[:].bitcast(U32), 0x5f3759e0,
                                        None, Alu.add)
                for _ in range(1):
                    t1q = tiny("t1q", 2, 4)
                    nc.vector.tensor_tensor(t1q[:], q[:], q[:], Alu.mult)
                    nc.vector.tensor_tensor(t1q[:], t1q[:], zq[:], Alu.mult)
                    nc.vector.tensor_scalar(t1q[:], t1q[:], -0.5, 1.5,
                                            Alu.mult, op1=Alu.add)
                    nc.vector.tensor_tensor(q[:], q[:], t1q[:], Alu.mult)

                t1 = tiny("t1", 2, 16)
                nc.vector.tensor_tensor(t1[:], A2[:], bc_n(q[:]), Alu.mult)
                OPt = tiny("OPt", 2, 16)
                nc.vector.scalar_tensor_tensor(OPt[:], t1[:], 0.5, bc_m(q[:]),
                                               Alu.mult, Alu.mult)
                col0 = reap(OPt[:], [[0, 4], [4, 4]])
                t2 = tiny("t2", 2, 16)
                nc.vector.tensor_tensor(t2[:], OPt[:], col0, Alu.mult)
                af = tiny("af", 2, 4)
                t2r = t2[:].rearrange("p (n m) -> p n m", n=4)
                nc.vector.tensor_reduce(af[:], t2r, mybir.AxisListType.X,
                                        Alu.add)
                al3 = tiny("al3", 2, 3)
                nc.vector.tensor_tensor(al3[:], af[:, 1:4], csb["cnrow"][:],
                                        Alu.mult)
                psb = ps_tiny.tile([128, 3], FP, tag="gt")
                nc.tensor.matmul(psb[:], csb["gbc"][:], al3[:],
                                 start=True, stop=True)
                aap = tiny("aap", 128, 3)
                nc.vector.tensor_copy(aap[:], psb[:])

                # alpha-scaled node weights; sw0 gates the next node phase,
                # so compute it on DVE straight from PSUM
                sw = [swpool.tile([128, 9 * 128], BF, tag=f"sw{n}",
                                  name=f"sw{n}") for n in range(3)]
                nc.vector.tensor_scalar_mul(
                    sw[0][:, 0:128], csb["wnod"][:, 0:128], aap[:, 0:1])
                nc.vector.tensor_scalar_mul(
                    sw[0][:, 128:9 * 128], csb["wnod"][:, 128:9 * 128],
                    aap[:, 0:1])
                if DBG and t == 0:
                    nc.sync.dma_start(dbg["d_aap"][:], aap[:])
                    nc.sync.dma_start(dbg["d_q"][:], q[:])
                for n in (1, 2):
                    nc.scalar.activation(
                        sw[n][:],
                        csb["wnod"][:, n * 9 * 128:(n + 1) * 9 * 128],
                        Act.Copy, scale=aap[:, n:n + 1])

                prev = (o0r, sw, f04, t)

    if not nc.is_finalized():
        nc.finalize()
    return nc


_NC_CACHE = {}


def _get_nc(yw, biasn_nz):
    key = (tuple(float(v) for v in yw), biasn_nz)
    if key not in _NC_CACHE:
        _NC_CACHE[key] = build_nc(yw, biasn_nz)
    return _NC_CACHE[key]


def _make_xpad(xcore):
    """[T,BC,CIN,64,64] f32 -> host im2col, flat bf16 [T*72*4096].

    Rows 0-35 of each step: bf16-hi of the zero-padded shifted x;
    rows 36-71: bf16 of the residual (x - hi). Row dy*12+dx*4+(b*2+ci)."""
    xp = np.zeros((T, 4, 66, 66), np.float32)
    xp[:, :, 1:65, 1:65] = xcore.reshape(T, 4, 64, 64)
    xim = np.empty((T, 72, 4096), ml_dtypes.bfloat16)
    for dy in range(3):
        for dx in range(3):
            blk = xp[:, :, dy:dy + 64, dx:dx + 64].reshape(T, 4, 4096)
            hi = blk.astype(ml_dtypes.bfloat16)
            lo = (blk - hi.astype(np.float32)).astype(ml_dtypes.bfloat16)
            r = dy * 12 + dx * 4
            xim[:, r:r + 4] = hi
            xim[:, 36 + r:36 + r + 4] = lo
    return np.ascontiguousarray(xim.reshape(-1))


def kernel(**inputs):
    x = np.asarray(inputs["x"], np.float32)
    consts, yw, (biasn_nz,) = _host_consts(
        inputs["conv0_w"], inputs["bn0_g"], inputs["bn0_b"], inputs["bn0_m"],
        inputs["bn0_v"], inputs["lif0_w"], inputs["convs_w"], inputs["bns_g"],
        inputs["bns_b"], inputs["bns_m"], inputs["bns_v"], inputs["lifs_w"],
        inputs["ft_w"], inputs["ft_b"], inputs["gat_w"], inputs["gat_a"],
        inputs["out_weights"])
    consts = {k: np.ascontiguousarray(v) for k, v in consts.items()}
    nc = _get_nc(yw, biasn_nz)
    core_ids = list(range(NCORES))
    in_maps = []
    for k in core_ids:
        m = dict(consts)
        m["xpad"] = _make_xpad(x[:, k * BC:(k + 1) * BC])
        in_maps.append(m)
    res = run_bass_kernel_spmd(nc, in_maps, core_ids).results
    out = np.concatenate([res[k]["y"] for k in core_ids], axis=1)
    return out.astype(np.float32)
